# revision 4
# baseline (speedup 1.0000x reference)
"""Causal self-attention (B=4, T=2048, D=1024, H=16) on 8 TRN2 NeuronCores.

Sharding: core c -> (batch b = c//2, head-group g = c%2 of 8 heads).
Host pre-transposes x to xT [D, T] bf16 and pre-casts/repacks weight slices
to bf16 (W_qkv carries the x T-quarter-0 block so the startup batch is one
DMA per 128-row chunk); the two out-proj partials per batch are summed on
the host (the tensor-parallel all-reduce), as in the original baseline.

Per-core program (identical SPMD on all 8 cores), TimelineSim ~233us vs the
296us baseline:
  1. DMA xT bf16 directly into SBUF (no on-device transpose) in T-quarter-
     major order; the first q/k/v projection chains run kk-interleaved over
     six shared PSUM slots so the PE tracks the upload front.
  2. qT/kT via W-stationary matmuls -> [512(feat), 2048(t)] bf16 (bias added
     on VectorE during the PSUM drain); V via xT-stationary matmuls at
     head-pair granularity -> vaug [2048(t), 8*65] bf16 with a ones column
     per head. V chains for head-pair h+1 are queued as filler during phase
     h, so every attention phase has projection work to absorb exp pacing.
  3. Per (head-pair, q-tile 512): scoresT [k,q] in PSUM (2 heads packed in
     the two 512-col halves), exp on ScalarE (scale=1/8, fp32->bf16),
     causal 0/1-mask multiply on diagonal tiles (VectorE). AV is FLIPPED:
     out[q(128), dv+1(65)] with the probs tile as lhsT (K=128, M=128 ->
     half the streamed columns of the [65, q] formulation). The two
     accumulators [128, 4*65] hold four q-chunks each as a SINGLE PSUM
     accumulation group (a matmul start zeroes the whole 2KB zone, so only
     one start/stop pair per bank is legal). AV emission lags the scores by
     8 k-tiles and weaves between score matmuls (PE is in-order; ready work
     must precede the instruction that stalls). Normalize = per-partition
     reciprocal + tensor_scalar multiply (no DRAM broadcast bounce); a
     small TensorE transpose returns attn to [dv, q] for the out-proj.
  4. out_proj: attnT-stationary matmuls vs W_out rows -> y partial, DMA out.
     At head-pair 3 each transpose is paired with the out-proj chain that
     consumes it in one FIFO queue (so it can never be overtaken); the last
     unit runs c-major with per-chunk accumulators so the tail is short.

Scheduling: a filler queue of projection/out-proj generator chains is
pumped between score matmuls under a per-phase ledger (PE-emitted ns vs
ScalarE-emitted ns + graduated lead), with deferred transposes interleaved
one-per-pump; phase-boundary drains alternate transposes and chains so the
two shared PSUM slots never serialize a transpose burst.
"""

from collections import deque

import numpy as np
import ml_dtypes

import concourse.bass as bass
import concourse.mybir as mybir
import concourse.tile as tile
from concourse.bass_utils import run_bass_kernel_spmd
from concourse.masks import make_identity

F32 = mybir.dt.float32
BF16 = mybir.dt.bfloat16
AX = mybir.AluOpType
EXP = mybir.ActivationFunctionType.Exp

T = 2048
D = 1024
HLOC = 8          # heads per core
QT = 512          # query tile
NQT = T // QT     # 4
NDIN = D // 128   # 8 contraction chunks
NMT = 4           # q/k feature m-tiles (512 local feats / 128)
VA = 65           # V cols per head incl. ones column
VB = HLOC * VA    # vaug cols per k-tile block
WC = 1536         # wbf cols per kk chunk

PE_NS = 0.4167    # ns per streamed matmul row (bf16, full p-state)
ACT_NS = 0.8333   # ns per activation col
ACT_FIX = 185.0   # per-activation-instruction fixed busy ns
LEAD_NS = 1400.0  # how far PE emission may run ahead of ScalarE emission


# W_qkv columns are host-repacked per kk chunk as:
#   [q mt0 (128) | k mt0 (128) | v (512) | q mt1..3 (384) | k mt1..3 (384)]
def _woff(sec, mt):
    if mt == 0:
        return sec * 128
    return 768 + sec * 384 + (mt - 1) * 128


# logical W col -> physical wx col (x-n0 occupies physical 768:1280)
def _wphys(off):
    return off if off < 768 else off + 512


_WOFF_V = 256

_NOP_ID = [0]


def _split_multiwaits(nc, limit=1):
    """This toolchain's walrus rejects more than one sync-wait on an
    instruction ("Too many sync wait commands"). Move excess waits onto
    same-engine NOPs inserted immediately before the instruction — the
    engine sequencer executes them in program order, so semantics are
    preserved (issue-after-wait implies execute-after-wait for DMA too)."""
    for f in nc.m.functions:
        for blk in f.blocks:
            new = []
            changed = False
            for inst in blk.instructions:
                si = inst.sync_info
                if si is not None and len(si.on_wait) > limit:
                    waits = list(si.on_wait)
                    inst.sync_info = mybir.SyncInfo(
                        on_wait=waits[:limit], on_update=list(si.on_update))
                    for w in waits[limit:]:
                        _NOP_ID[0] += 1
                        nop = mybir.InstNoOp(
                            name=f"waitnop-{_NOP_ID[0]}", ins=[], outs=[])
                        nop.engine = inst.engine
                        nop.sync_info = mybir.SyncInfo(on_wait=[w], on_update=[])
                        new.append(nop)
                    changed = True
                new.append(inst)
            if changed:
                blk.instructions = new


def build_nc():
    nc = bass.Bass()
    x_ext = nc.declare_dram_parameter("x", [D, T - 512], BF16, isOutput=False)
    w_ext = nc.declare_dram_parameter("W_qkv", [D, 2048], BF16, isOutput=False)
    b_ext = nc.declare_dram_parameter("b_qkv", [WC], F32, isOutput=False)
    wo_ext = nc.declare_dram_parameter("W_out", [512, D], BF16, isOutput=False)
    out_ext = nc.declare_dram_parameter("out", [T, D], F32, isOutput=True)

    pe_ns = [0.0]
    act_ns = [0.0]
    pe_base = [0.0]
    act_base = [0.0]

    with tile.TileContext(nc) as tc:
        with (
            tc.tile_pool(name="const", bufs=1) as constp,
            tc.tile_pool(name="big", bufs=1) as bigp,
        ):
            identb = constp.tile([128, 128], BF16, tag="identb")
            make_identity(nc, identb)

            # causal 0/1 mask [128, 2x512] bf16 (head-duplicated triangle):
            # mask[p, (h, f)] = 1 if f >= p else 0.
            maskt = constp.tile([128, 1024], BF16, tag="maskt")
            nc.gpsimd.memset(maskt, 1.0)
            mk3 = maskt.rearrange("p (h f) -> p h f", f=512)
            nc.gpsimd.affine_select(
                out=mk3, in_=mk3,
                compare_op=AX.is_ge, fill=0.0,
                base=0, channel_multiplier=-1,
                pattern=[[0, 2], [1, 512]],
            )

            bq_sb = constp.tile([128, NMT], F32, tag="bq")
            bk_sb = constp.tile([128, NMT], F32, tag="bk")
            bv_sb = constp.tile([128, 512], F32, tag="bv")

            # persistent SBUF tensors
            wx = bigp.tile([128, NDIN * 2048], BF16, tag="wx")
            xT = bigp.tile([128, NDIN * (T - 512)], BF16, tag="xT")
            woutb = bigp.tile([128, 4 * D], BF16, tag="woutb")
            qT = bigp.tile([128, NMT * T], BF16, tag="qT")
            kT = bigp.tile([128, NMT * T], BF16, tag="kT")
            vaug = bigp.tile([128, (T // 128) * VB], BF16, tag="vaug")
            attnT = bigp.tile([128, 4 * T], BF16, tag="attnT")

            def wap(kk, off, width):
                base = kk * 2048 + _wphys(off)
                return wx[:, base: base + width]

            def xap(kk, n):
                # x T-quarter n: n0 lives inside wx; n1..3 in xT
                if n == 0:
                    return wx[:, kk * 2048 + 768: kk * 2048 + 1280]
                return xT[:, kk * 1536 + (n - 1) * 512: kk * 1536 + n * 512]

            def xap128(kk, tt):
                # x T-column block of 128 at tile tt (for v-chain lhsT)
                n, r = divmod(tt, 4)
                return xap(kk, n)[:, r * 128:(r + 1) * 128]

            # ---- input DMAs, priority order -------------------------------
            # W part 1 (qk mt0 + v) and x quarter 0, kk-interleaved; the
            # bias vectors (needed at the first drains ~8us in) slot in
            # after the first few pairs so they don't delay the first matmul
            for kk in range(NDIN):
                nc.sync.dma_start(
                    out=wx[:, kk * 2048: kk * 2048 + 1280],
                    in_=w_ext[kk * 128:(kk + 1) * 128, 0:1280])
                if kk == 2:
                    nc.sync.dma_start(
                        out=bq_sb,
                        in_=b_ext[0:512].rearrange("(m p) -> p m", p=128))
                    nc.sync.dma_start(
                        out=bk_sb,
                        in_=b_ext[512:1024].rearrange("(m p) -> p m", p=128))
                if kk == 5:
                    bv_src = b_ext[1024:WC]
                    nc.sync.dma_start(
                        out=bv_sb,
                        in_=bass.AP(tensor=bv_src.tensor, offset=bv_src.offset,
                                    ap=[[0, 128]] + list(bv_src.ap)),
                    )
            # x quarter 1 (needed by the qt1 JIT projections ~12us in);
            # W part 2 (qk mt1-3) is only consumed from hp1 on, so it can
            # follow; then x quarters 2..3
            for kk in range(NDIN):
                nc.sync.dma_start(
                    out=xT[:, kk * 1536: kk * 1536 + 512],
                    in_=x_ext[kk * 128:(kk + 1) * 128, 0:512])
            for kk in range(NDIN):
                nc.sync.dma_start(
                    out=wx[:, kk * 2048 + 1280:(kk + 1) * 2048],
                    in_=w_ext[kk * 128:(kk + 1) * 128, 1280:2048])
            for n in range(2, NQT):
                for kk in range(NDIN):
                    nc.sync.dma_start(
                        out=xT[:, kk * 1536 + (n - 1) * 512:
                               kk * 1536 + n * 512],
                        in_=x_ext[kk * 128:(kk + 1) * 128,
                                  (n - 1) * 512: n * 512])
            for k4 in range(4):
                nc.sync.dma_start(
                    out=woutb[:, k4 * D:(k4 + 1) * D],
                    in_=wo_ext[k4 * 128:(k4 + 1) * 128, :])

            with (
                tc.tile_pool(name="scps", bufs=2, space="PSUM") as scps,
                tc.tile_pool(name="avps", bufs=2, space="PSUM") as avps,
                tc.tile_pool(name="pjpsum", bufs=2, space="PSUM") as pjpsum,
                tc.tile_pool(name="ptp", bufs=24) as ptp,
                tc.tile_pool(name="recp", bufs=4) as recp,
                tc.tile_pool(name="savp", bufs=14) as savp,
                tc.tile_pool(name="yo", bufs=3) as yo,
            ):
                def qk_drain(ps, sec, mt, n):
                    dst, bias = ((qT, bq_sb), (kT, bk_sb))[sec]
                    nc.vector.tensor_scalar_add(
                        out=dst[:, mt * T + n * 512: mt * T + (n + 1) * 512],
                        in0=ps, scalar1=bias[:, mt:mt + 1])

                def v_drain(ps, tt, vhp):
                    blk3 = vaug[:, tt * VB:(tt + 1) * VB].rearrange(
                        "p (h c) -> p h c", c=VA)
                    bv3 = bv_sb.rearrange("p (h c) -> p h c", c=64)
                    nc.vector.tensor_tensor(
                        out=blk3[:, 2 * vhp:2 * vhp + 2, 0:64],
                        in0=ps.rearrange("p (h c) -> p h c", c=64),
                        in1=bv3[:, 2 * vhp:2 * vhp + 2, :],
                        op=AX.add)
                    nc.vector.memset(
                        blk3[:, 2 * vhp:2 * vhp + 2, 64:65], 1.0)

                def gen_qkchain(sec, mt, n):
                    ps = pjpsum.tile([128, 512], F32, tag="pj", name="pj")
                    off = _woff(sec, mt)
                    for kk in range(NDIN):
                        nc.tensor.matmul(
                            ps,
                            lhsT=wap(kk, off, 128),
                            rhs=xap(kk, n),
                            start=(kk == 0), stop=(kk == NDIN - 1),
                        )
                        pe_ns[0] += 512 * PE_NS
                        if kk % 2 == 1 and kk < NDIN - 1:
                            yield
                    qk_drain(ps, sec, mt, n)

                def gen_vchain(tt, vhp):
                    ps = pjpsum.tile([128, 128], F32, tag="pj", name="pv")
                    off = _WOFF_V + vhp * 128
                    for kk in range(NDIN):
                        nc.tensor.matmul(
                            ps,
                            lhsT=xap128(kk, tt),
                            rhs=wap(kk, off, 128),
                            start=(kk == 0), stop=(kk == NDIN - 1),
                        )
                        pe_ns[0] += 128 * PE_NS
                        if kk == 3:
                            yield
                    v_drain(ps, tt, vhp)

                def emit_jit_quartet(chains):
                    """First projections, kk-interleaved over up to 4 PSUM
                    slots (2 borrowed from the scores pool) so the PE keeps
                    pace with the chunked x/W upload instead of stalling on
                    the last chunk of each chain."""
                    slots = [(scps, "sc"), (scps, "sc"), (pjpsum, "pj"),
                             (pjpsum, "pj"), (avps, "av"), (avps, "av")]
                    tiles = []
                    for idx, ch in enumerate(chains):
                        pool, tg = slots[idx]
                        shape = [128, 512] if ch[0] == "qk" else [128, 128]
                        tiles.append(pool.tile(shape, F32, tag=tg,
                                               name="jit"))
                    for kk in range(NDIN):
                        for ps, ch in zip(tiles, chains):
                            if ch[0] == "qk":
                                _, sec, mt, n = ch
                                nc.tensor.matmul(
                                    ps,
                                    lhsT=wap(kk, _woff(sec, mt), 128),
                                    rhs=xap(kk, n),
                                    start=(kk == 0), stop=(kk == NDIN - 1),
                                )
                                pe_ns[0] += 512 * PE_NS
                            else:
                                _, tt, vhp = ch
                                nc.tensor.matmul(
                                    ps,
                                    lhsT=xap128(kk, tt),
                                    rhs=wap(kk, _WOFF_V + vhp * 128, 128),
                                    start=(kk == 0), stop=(kk == NDIN - 1),
                                )
                                pe_ns[0] += 128 * PE_NS
                    for ps, ch in zip(tiles, chains):
                        if ch[0] == "qk":
                            qk_drain(ps, ch[1], ch[2], ch[3])
                        else:
                            v_drain(ps, ch[1], ch[2])

                def emit_ychain(mt, n, tail=False):
                    ps = pjpsum.tile([128, 512], F32, tag="pj", name="py")
                    for kk in range(4):
                        nc.tensor.matmul(
                            ps,
                            lhsT=attnT[:, kk * T + mt * 128:
                                       kk * T + (mt + 1) * 128],
                            rhs=woutb[:, kk * D + n * 512:
                                      kk * D + (n + 1) * 512],
                            start=(kk == 0), stop=(kk == 3))
                    yt = yo.tile([128, 512], F32, tag="yt")
                    (nc.scalar.copy if tail else nc.vector.tensor_copy)(yt, ps)
                    nc.sync.dma_start(
                        out=out_ext[mt * 128:(mt + 1) * 128,
                                    n * 512:(n + 1) * 512], in_=yt)
                    pe_ns[0] += 4 * 512 * PE_NS

                def gen_ychain_pair(mt):
                    for n in range(2):
                        ps = pjpsum.tile([128, 512], F32, tag="pj", name="py")
                        for kk in range(4):
                            nc.tensor.matmul(
                                ps,
                                lhsT=attnT[:, kk * T + mt * 128:
                                           kk * T + (mt + 1) * 128],
                                rhs=woutb[:, kk * D + n * 512:
                                          kk * D + (n + 1) * 512],
                                start=(kk == 0), stop=(kk == 3))
                            pe_ns[0] += 512 * PE_NS
                            if kk % 2 == 1 and kk < 3:
                                yield
                        yt = yo.tile([128, 512], F32, tag="yt")
                        nc.vector.tensor_copy(yt, ps)
                        nc.sync.dma_start(
                            out=out_ext[mt * 128:(mt + 1) * 128,
                                        n * 512:(n + 1) * 512], in_=yt)
                        yield

                fillq = deque()
                tpq = deque()

                def pump_tp():
                    if tpq:
                        tpq.popleft()()

                active = [None]

                def ahead():
                    return ((pe_ns[0] - pe_base[0])
                            - (act_ns[0] - act_base[0]))

                def step_fill():
                    # one generator step (or start the next queued filler);
                    # returns False when no filler work exists
                    if active[0] is None:
                        if not fillq:
                            return False
                        active[0] = iter(fillq.popleft()())
                    try:
                        next(active[0])
                    except StopIteration:
                        active[0] = None
                        return step_fill()
                    return True

                def finish_active():
                    # run the suspended filler chain to completion — required
                    # before anything else may allocate a "pj"-tag PSUM slot,
                    # or the pool rotation would alias the chain's live
                    # accumulator and corrupt it
                    while active[0] is not None:
                        try:
                            next(active[0])
                        except StopIteration:
                            active[0] = None

                lead = [LEAD_NS]

                def pump_one(hard=False):
                    if tpq and active[0] is None \
                            and (hard or ahead() < lead[0]):
                        tpq.popleft()()
                    while (hard or ahead() < lead[0]):
                        if not step_fill():
                            break

                def pump(hard=False):
                    while tpq and active[0] is None \
                            and (hard or ahead() < LEAD_NS):
                        tpq.popleft()()
                    while (hard or ahead() < LEAD_NS):
                        if not step_fill():
                            break

                def drain_fill():
                    # interleave transposes with chain work so the 2 shared
                    # PSUM slots never serialize a burst of transposes
                    while True:
                        if tpq and active[0] is None:
                            tpq.popleft()()
                        progressed = False
                        for _ in range(6):
                            if step_fill():
                                progressed = True
                            else:
                                break
                        if not tpq and not fillq and active[0] is None:
                            break
                        if not progressed and not tpq:
                            break
                    finish_active()

                m3 = maskt.rearrange("p (h q) -> p h q", q=512)

                def gen_tp(sav2, hp, qt, c):
                    emit_transp(sav2, hp, qt, c)
                    return
                    yield

                def emit_transp(sav2, hp, qt, c):
                    tp = pjpsum.tile([128, 128], BF16, tag="pj", name="tp")
                    nc.tensor.transpose(tp, sav2, identb)
                    pe_ns[0] += 128 * PE_NS
                    nc.vector.tensor_copy(
                        attnT[:, hp * T + qt * 512 + c * 128:
                              hp * T + qt * 512 + (c + 1) * 128],
                        tp)

                def emit_unit(hp, qt, pump_hard=False, inline_out=False):
                    nkt = 4 * (qt + 1)
                    acc = None
                    if not inline_out:
                        acc = [avps.tile([128, 4 * VA], F32, tag="av",
                                         name=f"av{hp}{qt}{par}")
                               for par in range(2)]

                    def emit_avs(kt, pt):
                        # one PSUM accumulation group per par-accumulator
                        # (a start zeroes the whole 2KB zone, so the four
                        # c-slices share a single start/stop pair)
                        for c in range(4):
                            last_kt = 4 * qt + c
                            if kt > last_kt:
                                continue
                            for par in range(2):
                                h = 2 * hp + par
                                nc.tensor.matmul(
                                    acc[par][:, c * VA:(c + 1) * VA],
                                    lhsT=pt[:, par * 512 + c * 128:
                                            par * 512 + (c + 1) * 128],
                                    rhs=vaug[:, kt * VB + h * VA:
                                             kt * VB + (h + 1) * VA],
                                    start=(kt == 0 and c == 0),
                                    stop=(kt == nkt - 1 and c == 3),
                                )
                                pe_ns[0] += VA * PE_NS

                    # scores master loop; the previous chunk's AV matmuls and
                    # pumped filler work are woven BETWEEN consecutive score
                    # matmuls so the PE always has ready work while exp paces
                    # the PSUM slot rotation (PE is in-order).
                    pend = []          # (kt, pt) awaiting AV emission
                    for kt in range(nkt):
                        if len(pend) > 8 and not inline_out:
                            k0, p0 = pend.pop(0)
                            emit_avs(k0, p0)
                        pump_one(hard=pump_hard)
                        i = kt - (nkt - 4)   # diagonal index (>=0: diag)
                        lo_q = max(0, 128 * i)
                        ps = scps.tile([128, 1024], F32, tag="sc", name="sc")
                        for par in range(2):
                            lo, hi = par * 64, par * 64 + 64
                            nc.tensor.matmul(
                                ps[:, par * 512 + lo_q:(par + 1) * 512],
                                lhsT=kT[lo:hi, hp * T + kt * 128:
                                        hp * T + (kt + 1) * 128],
                                rhs=qT[lo:hi, hp * T + qt * 512 + lo_q:
                                       hp * T + (qt + 1) * 512],
                                start=True, stop=True,
                            )
                            pe_ns[0] += (512 - lo_q) * PE_NS
                        pt = ptp.tile([128, 1024], BF16, tag="pt")
                        ps3 = ps.rearrange("p (h q) -> p h q", q=512)
                        pt3 = pt.rearrange("p (h q) -> p h q", q=512)
                        nc.scalar.activation(
                            pt3[:, :, lo_q:512], ps3[:, :, lo_q:512],
                            EXP, bias=0.0, scale=0.125)
                        act_ns[0] += 2 * (512 - lo_q) * ACT_NS + ACT_FIX
                        if i >= 0:
                            nc.vector.tensor_tensor(
                                out=pt3[:, :, lo_q:512],
                                in0=pt3[:, :, lo_q:512],
                                in1=m3[:, :, 0:512 - lo_q], op=AX.mult)
                        pend.append((kt, pt))
                    if inline_out:
                        # last unit: c-major per-chunk accumulators in the
                        # two alternating PSUM slots; each chunk's normalize,
                        # transpose and out-proj fire while the next chunk's
                        # AV chain runs — the tail shrinks to one chunk
                        pump(hard=True)
                        for c in range(4):
                            last_kt = 4 * qt + c
                            ac = [avps.tile([128, VA], F32, tag="av",
                                            name=f"avt{c}{par}")
                                  for par in range(2)]
                            for kt, pt in pend:
                                if kt > last_kt:
                                    continue
                                for par in range(2):
                                    h = 2 * hp + par
                                    nc.tensor.matmul(
                                        ac[par],
                                        lhsT=pt[:, par * 512 + c * 128:
                                                par * 512 + (c + 1) * 128],
                                        rhs=vaug[:, kt * VB + h * VA:
                                                 kt * VB + (h + 1) * VA],
                                        start=(kt == 0), stop=(kt == last_kt),
                                    )
                                    pe_ns[0] += VA * PE_NS
                            sav2 = savp.tile([128, 128], BF16, tag="sav")
                            for par in range(2):
                                rc = recp.tile([128, 1], F32, tag="rc")
                                nc.vector.reciprocal(rc, ac[par][:, 64:65])
                                nc.vector.tensor_scalar_mul(
                                    out=sav2[:, par * 64:(par + 1) * 64],
                                    in0=ac[par][:, 0:64], scalar1=rc)
                            emit_transp(sav2, hp, qt, c)
                            mt = qt * 4 + c
                            emit_ychain(mt, 0, tail=(c == 3))
                            emit_ychain(mt, 1, tail=(c == 3))
                        return
                    for k0, p0 in pend:
                        pump_one(hard=pump_hard)
                        emit_avs(k0, p0)
                    # normalize (VectorE) eagerly — frees the PSUM
                    # accumulators; transposes are deferred so they
                    # interleave with later PE work instead of serializing
                    # behind the VectorE here.
                    for c in range(4):
                        sav2 = savp.tile([128, 128], BF16, tag="sav")
                        for par in range(2):
                            rc = recp.tile([128, 1], F32, tag="rc")
                            nc.vector.reciprocal(
                                rc, acc[par][:, c * VA + 64: c * VA + 65])
                            nc.vector.tensor_scalar_mul(
                                out=sav2[:, par * 64:(par + 1) * 64],
                                in0=acc[par][:, c * VA: c * VA + 64],
                                scalar1=rc)
                        if hp == 3:
                            # pair the transpose with the out-proj chain that
                            # consumes its attnT block, in one FIFO queue, so
                            # the chain can never overtake its producer
                            fillq.append(
                                lambda s=sav2, h2=hp, q2=qt, cc=c:
                                gen_tp(s, h2, q2, cc))
                            fillq.append(
                                lambda m=qt * 4 + c: gen_ychain_pair(m))
                        else:
                            tpq.append(
                                lambda s=sav2, h2=hp, q2=qt, cc=c:
                                emit_transp(s, h2, q2, cc))

                for hp in range(HLOC // 2):
                    if hp >= 1:
                        # hp's q/k projections must be fully emitted before
                        # its units (PE is in-order; a unit ahead of its
                        # producers would deadlock).
                        drain_fill()
                        if hp + 1 < 4:
                            # v(hp+1) is only consumed from hp+1 on, so it
                            # can weave into THIS phase's attention instead
                            # of burning in the next forced drain
                            for sec in range(2):
                                for n in range(NQT):
                                    fillq.append(
                                        lambda s=sec, m=hp + 1, nn=n:
                                        gen_qkchain(s, m, nn))
                            for tt in range(16):
                                fillq.append(
                                    lambda t2=tt, m=hp + 1:
                                    gen_vchain(t2, m))
                    else:
                        # queue hp1's projections; pumped during hp0 units
                        for sec in range(2):
                            for n in range(NQT):
                                fillq.append(
                                    lambda s=sec, nn=n:
                                    gen_qkchain(s, 1, nn))
                    pe_base[0] = pe_ns[0]
                    act_base[0] = act_ns[0]
                    for qt in range(NQT):
                        if hp == 0:
                            # JIT projections for this q-tile, interleaved;
                            # v chains for head-pairs 0 AND 1 ride along so
                            # the PE tracks the front of the x/W upload
                            finish_active()
                            t0 = 4 * qt
                            emit_jit_quartet([
                                ("qk", 0, 0, qt), ("qk", 1, 0, qt),
                                ("v", t0, 0), ("v", t0, 1),
                                ("v", t0 + 1, 0), ("v", t0 + 1, 1)])
                            emit_jit_quartet([
                                ("v", t0 + 2, 0), ("v", t0 + 2, 1),
                                ("v", t0 + 3, 0), ("v", t0 + 3, 1)])
                        last = (hp == 3 and qt == 3)
                        lead[0] = (400.0, 800.0, 1400.0, 3000.0)[qt]
                        emit_unit(hp, qt, pump_hard=last, inline_out=last)
                drain_fill()

    _split_multiwaits(nc)
    return nc


_NC_CACHE = {}


def get_nc():
    if "nc" not in _NC_CACHE:
        _NC_CACHE["nc"] = build_nc()
    return _NC_CACHE["nc"]


def _pack_w(wslice, xT_b):
    """Per-core combined upload buffer [1024, 2048] per 128-row chunk:
    [q mt0 | k mt0 | v | x T-quarter 0 | q mt1-3 | k mt1-3] — the first
    1280 columns are the startup batch (one DMA per chunk)."""
    wq, wk, wv = wslice[:, 0:512], wslice[:, 512:1024], wslice[:, 1024:1536]
    return np.concatenate(
        [wq[:, 0:128], wk[:, 0:128], wv, xT_b[:, 0:512],
         wq[:, 128:512], wk[:, 128:512]], axis=1)


def _pack_b(bslice):
    """b_qkv layout stays [q 512 | k 512 | v 512] (loads don't need repack)."""
    return bslice


def make_in_maps(x, W_qkv, b_qkv, W_out):
    bf16 = ml_dtypes.bfloat16
    in_maps = []
    for c in range(8):
        b, g = c // 2, c % 2
        s = slice(512 * g, 512 * (g + 1))
        wslice = np.concatenate(
            [W_qkv[:, 512 * g:512 * (g + 1)],
             W_qkv[:, 1024 + 512 * g:1024 + 512 * (g + 1)],
             W_qkv[:, 2048 + 512 * g:2048 + 512 * (g + 1)]], axis=1)
        bslice = np.concatenate(
            [b_qkv[512 * g:512 * (g + 1)],
             b_qkv[1024 + 512 * g:1024 + 512 * (g + 1)],
             b_qkv[2048 + 512 * g:2048 + 512 * (g + 1)]])
        xT_b = np.ascontiguousarray(x[b].T).astype(bf16)
        wfull = np.ascontiguousarray(
            _pack_w(wslice.astype(np.float32), xT_b.astype(np.float32)))
        in_maps.append({
            "x": np.ascontiguousarray(xT_b[:, 512:2048]),
            "W_qkv": wfull.astype(bf16),
            "b_qkv": np.ascontiguousarray(_pack_b(bslice), dtype=np.float32),
            "W_out": np.ascontiguousarray(W_out[s]).astype(bf16),
        })
    return in_maps


def kernel(x, W_qkv, b_qkv, W_out, b_out):
    x = np.asarray(x)
    W_qkv = np.asarray(W_qkv)
    b_qkv = np.asarray(b_qkv)
    W_out = np.asarray(W_out)
    b_out = np.asarray(b_out)
    nc = get_nc()
    in_maps = make_in_maps(x, W_qkv, b_qkv, W_out)
    res = run_bass_kernel_spmd(nc, in_maps, core_ids=list(range(8))).results
    out = np.stack(
        [res[2 * b]["out"] + res[2 * b + 1]["out"] for b in range(4)], axis=0)
    out = out + b_out[None, None, :]
    return out.astype(np.float32)
